# revision 1
# baseline (speedup 1.0000x reference)
import sys
import numpy as np

sys.path.insert(0, '/opt/trn_rl_repo')

import concourse.bass as bass
import concourse.bacc as bacc
import concourse.tile as tile
from concourse import mybir
from concourse.bass_utils import run_bass_kernel_spmd
from contextlib import ExitStack

F32 = mybir.dt.float32
F32R = mybir.dt.float32r

B, S, HID = 2, 4096, 4096
NH, HD = 16, 256
RD = 64
THETA = 10000.0
NKMAX = 8          # max k-chunks of 512 per q-tile row
NEG = -1.0e30

_cached = {}


def _build_program():
    nc = bacc.Bacc("TRN2", target_bir_lowering=False, debug=False, num_devices=8)
    # hidden, transposed and swizzled host-side into contiguous 2MB col-blocks:
    # hsw[st] = hiddenT[:, st*128:(st+1)*128]
    hiddenT = nc.declare_dram_parameter("hiddenT", [32, HID, 128], F32R,
                                        isOutput=False)
    wqkvT = nc.declare_dram_parameter("wqkvT", [HID, 3072], F32R, isOutput=False)
    woutTp = nc.declare_dram_parameter("woutTp", [HID, 1024], F32R, isOutput=False)
    cs_e = nc.declare_dram_parameter("cs", [S, 32], F32, isOutput=False)
    sn_e = nc.declare_dram_parameter("sn", [S, 32], F32, isOutput=False)
    msk_e = nc.declare_dram_parameter("msk", [128, 4, 512], F32, isOutput=False)
    id_e = nc.declare_dram_parameter("ident", [128, 128], F32R, isOutput=False)
    out_e = nc.declare_dram_parameter("out", [S, 1024], F32, isOutput=True)

    Copy = mybir.ActivationFunctionType.Copy
    Exp = mybir.ActivationFunctionType.Exp
    AX = mybir.AxisListType.X

    with tile.TileContext(nc) as tc:
        with tc.tile_pool(name="dram", bufs=1, space="DRAM") as dram:
            qs = dram.tile([S, 1024], F32R)
            ks = dram.tile([S, 1024], F32R)
            vs = dram.tile([S, 1024], F32R)
            at_h = [dram.tile([256, S], F32R, name=f"at{j}") for j in range(4)]
            gt_h = [dram.tile([1024, S], F32R, name=f"gt{j}") for j in range(4)]

            # ---------------- phase 1: QKV projection + RoPE ----------------
            with ExitStack() as s1:
                wpool = s1.enter_context(tc.tile_pool(name="wq", bufs=1))
                hpool = s1.enter_context(tc.tile_pool(name="hid", bufs=2))
                evpool = s1.enter_context(tc.tile_pool(name="ev", bufs=4))
                cpool = s1.enter_context(tc.tile_pool(name="cspool", bufs=2))
                tpool = s1.enter_context(tc.tile_pool(name="ropetmp", bufs=4))
                pq = s1.enter_context(tc.tile_pool(name="pq", bufs=2, space="PSUM"))
                hviews = hiddenT.ap().rearrange("t (ho p) s -> t p ho s", p=128)
                for wb in range(3):
                    wt = []
                    for h in range(32):
                        w_t = wpool.tile([128, 1024], F32R, name=f"w{h}", tag=f"w{h}")
                        nc.sync.dma_start(
                            out=w_t,
                            in_=wqkvT.ap()[h * 128:(h + 1) * 128,
                                           wb * 1024:(wb + 1) * 1024])
                        wt.append(w_t)
                    for st in range(32):
                        hs = hpool.tile([128, 32, 128], F32R, name="hs")
                        nc.sync.dma_start(out=hs, in_=hviews[st])
                        if wb < 2:
                            ct = cpool.tile([128, 32], F32, name="ct")
                            snt = cpool.tile([128, 32], F32, name="snt")
                            nc.sync.dma_start(
                                out=ct, in_=cs_e.ap()[st * 128:(st + 1) * 128, :])
                            nc.sync.dma_start(
                                out=snt, in_=sn_e.ap()[st * 128:(st + 1) * 128, :])
                        for oc in range(2):
                            ps = pq.tile([128, 512], F32, name="qkps")
                            for h in range(32):
                                nc.tensor.matmul(
                                    ps, hs[:, h, :],
                                    wt[h][:, oc * 512:(oc + 1) * 512],
                                    start=(h == 0), stop=(h == 31))
                            ev = evpool.tile([128, 512], F32R, name="ev")
                            if wb < 2:
                                for hb in range(2):
                                    b0 = hb * 256
                                    x1 = ps[:, b0 + 0:b0 + 64:2]
                                    x2 = ps[:, b0 + 1:b0 + 65:2]
                                    ta = tpool.tile([128, 32], F32, name="ta")
                                    tb = tpool.tile([128, 32], F32, name="tb")
                                    nc.vector.tensor_mul(ta, x1, ct)
                                    nc.vector.tensor_mul(tb, x2, snt)
                                    nc.vector.tensor_sub(ev[:, b0:b0 + 32], ta, tb)
                                    tc2 = tpool.tile([128, 32], F32, name="tc2")
                                    td = tpool.tile([128, 32], F32, name="td")
                                    nc.vector.tensor_mul(tc2, x2, ct)
                                    nc.vector.tensor_mul(td, x1, snt)
                                    nc.vector.tensor_add(
                                        ev[:, b0 + 32:b0 + 64], tc2, td)
                                    nc.scalar.activation(
                                        ev[:, b0 + 64:b0 + 256],
                                        ps[:, b0 + 64:b0 + 256], Copy)
                            else:
                                nc.scalar.activation(ev, ps, Copy)
                            dst = (qs, ks, vs)[wb]
                            nc.sync.dma_start(
                                out=dst[st * 128:(st + 1) * 128,
                                        oc * 512:(oc + 1) * 512],
                                in_=ev)

            # ---------------- phase 2: attention per head + gather ----------
            with ExitStack() as s2:
                kv = s2.enter_context(tc.tile_pool(name="kv", bufs=1))
                scp = s2.enter_context(tc.tile_pool(name="scp", bufs=1))
                small = s2.enter_context(tc.tile_pool(name="small", bufs=4))
                ptp = s2.enter_context(tc.tile_pool(name="ptp", bufs=6))
                consts = s2.enter_context(tc.tile_pool(name="consts", bufs=1))
                pst = s2.enter_context(tc.tile_pool(name="pst", bufs=2, space="PSUM"))
                pso = s2.enter_context(tc.tile_pool(name="pso", bufs=2, space="PSUM"))
                idt = consts.tile([128, 128], F32R)
                nc.sync.dma_start(out=idt, in_=id_e.ap())
                mskt = consts.tile([128, 4, 512], F32)
                nc.sync.dma_start(out=mskt, in_=msk_e.ap())
                vviews = vs.rearrange("(st p) o -> p st o", p=128)
                for h in range(4):
                    KT = [kv.tile([128, S], F32R, name=f"kt{d}", tag=f"kt{d}")
                          for d in range(2)]
                    QT = [kv.tile([128, S], F32R, name=f"qt{d}", tag=f"qt{d}")
                          for d in range(2)]
                    for st in range(32):
                        kin = ptp.tile([128, 256], F32R, name="kin")
                        nc.sync.dma_start(
                            out=kin, in_=ks[st * 128:(st + 1) * 128,
                                            h * 256:(h + 1) * 256])
                        qin = ptp.tile([128, 256], F32R, name="qin")
                        nc.sync.dma_start(
                            out=qin, in_=qs[st * 128:(st + 1) * 128,
                                            h * 256:(h + 1) * 256])
                        for d in range(2):
                            tpk = pst.tile([128, 128], F32R, name="tprs", tag="tprs")
                            nc.tensor.transpose(tpk, kin[:, d * 128:(d + 1) * 128], idt)
                            nc.vector.tensor_copy(
                                KT[d][:, st * 128:(st + 1) * 128], tpk)
                            tpq = pst.tile([128, 128], F32R, name="tprs", tag="tprs")
                            nc.tensor.transpose(tpq, qin[:, d * 128:(d + 1) * 128], idt)
                            nc.vector.tensor_copy(
                                QT[d][:, st * 128:(st + 1) * 128], tpq)
                    vt = kv.tile([128, 32, 256], F32R, name="vt", tag="vt")
                    nc.sync.dma_start(
                        out=vt, in_=vviews[:, :, h * 256:(h + 1) * 256])
                    for qi in range(32):
                        nk = qi // 4 + 1
                        srow = scp.tile([128, S], F32, name="srow", tag="srow")
                        prow = scp.tile([128, S], F32R, name="prow", tag="prow")
                        for kc in range(nk):
                            pss = pst.tile([128, 512], F32, name="spsum", tag="spsum")
                            for d in range(2):
                                nc.tensor.matmul(
                                    pss, QT[d][:, qi * 128:(qi + 1) * 128],
                                    KT[d][:, kc * 512:(kc + 1) * 512],
                                    start=(d == 0), stop=(d == 1))
                            if kc == nk - 1:
                                nc.vector.tensor_add(
                                    srow[:, kc * 512:(kc + 1) * 512], pss,
                                    mskt[:, qi % 4, :])
                            else:
                                nc.scalar.activation(
                                    srow[:, kc * 512:(kc + 1) * 512], pss, Copy)
                        nmx = small.tile([128, 1], F32, name="nmx")
                        nc.vector.reduce_max(nmx, srow[:, 0:nk * 512],
                                             axis=AX, negate=True)
                        bia = small.tile([128, 1], F32, name="bia")
                        nc.vector.tensor_scalar_mul(bia, nmx, 1.0 / 16.0)
                        sums = small.tile([128, NKMAX], F32, name="sums")
                        for kc in range(nk):
                            nc.scalar.activation(
                                prow[:, kc * 512:(kc + 1) * 512],
                                srow[:, kc * 512:(kc + 1) * 512], Exp,
                                bias=bia, scale=1.0 / 16.0,
                                accum_out=sums[:, kc:kc + 1])
                        ssum = small.tile([128, 1], F32, name="ssum")
                        nc.vector.reduce_sum(ssum, sums[:, 0:nk], axis=AX)
                        rinv = small.tile([128, 1], F32, name="rinv")
                        nc.vector.reciprocal(rinv, ssum)
                        pot = pso.tile([128, 256], F32, name="opsum")
                        for kc in range(nk):
                            for t4 in range(4):
                                g = kc * 4 + t4
                                tpp = pst.tile([128, 128], F32R,
                                               name="tprs", tag="tprs")
                                nc.tensor.transpose(
                                    tpp, prow[:, g * 128:(g + 1) * 128], idt)
                                pts = ptp.tile([128, 128], F32R, name="pts")
                                nc.vector.tensor_copy(pts, tpp)
                                nc.tensor.matmul(
                                    pot, pts, vt[:, g, :],
                                    start=(g == 0), stop=(g == nk * 4 - 1))
                        att = ptp.tile([128, 256], F32R, name="att")
                        nc.vector.tensor_scalar_mul(att, pot, rinv)
                        for d in range(2):
                            tpa = pst.tile([128, 128], F32R, name="tprs", tag="tprs")
                            nc.tensor.transpose(
                                tpa, att[:, d * 128:(d + 1) * 128], idt)
                            ats = ptp.tile([128, 128], F32R, name="ats")
                            nc.vector.tensor_copy(ats, tpa)
                            nc.sync.dma_start(
                                out=at_h[h][d * 128:(d + 1) * 128,
                                            qi * 128:(qi + 1) * 128],
                                in_=ats)
                    nc.gpsimd.collective_compute(
                        "AllGather", mybir.AluOpType.bypass,
                        replica_groups=[[0, 1, 2, 3], [4, 5, 6, 7]],
                        ins=[at_h[h][:]], outs=[gt_h[h][:]])

            # ---------------- phase 3: output projection --------------------
            with ExitStack() as s3:
                wo = s3.enter_context(tc.tile_pool(name="wo", bufs=1))
                ga = s3.enter_context(tc.tile_pool(name="ga", bufs=2))
                ob = s3.enter_context(tc.tile_pool(name="ob", bufs=3))
                pout = s3.enter_context(tc.tile_pool(name="pout", bufs=2, space="PSUM"))
                wot = []
                for hh in range(32):
                    w_o = wo.tile([128, 1024], F32R, name=f"wo{hh}", tag=f"wo{hh}")
                    nc.sync.dma_start(
                        out=w_o, in_=woutTp.ap()[hh * 128:(hh + 1) * 128, :])
                    wot.append(w_o)
                gviews = [g.rearrange("(ho p) s -> p ho s", p=128) for g in gt_h]
                for st in range(32):
                    acb = [ga.tile([128, 8, 128], F32R, name=f"acb{j}", tag=f"acb{j}")
                           for j in range(4)]
                    for j in range(4):
                        nc.sync.dma_start(
                            out=acb[j],
                            in_=gviews[j][:, :, st * 128:(st + 1) * 128])
                    for oc in range(2):
                        po2 = pout.tile([128, 512], F32, name="po2")
                        for j in range(4):
                            for ht in range(8):
                                nc.tensor.matmul(
                                    po2, acb[j][:, ht, :],
                                    wot[j * 8 + ht][:, oc * 512:(oc + 1) * 512],
                                    start=(j == 0 and ht == 0),
                                    stop=(j == 3 and ht == 7))
                        osb = ob.tile([128, 512], F32, name="osb")
                        nc.scalar.activation(osb, po2, Copy)
                        nc.sync.dma_start(
                            out=out_e.ap()[st * 128:(st + 1) * 128,
                                           oc * 512:(oc + 1) * 512],
                            in_=osb)

    nc.compile()
    return nc


def kernel(hidden_states, position_ids, Wqkv, Wout):
    hidden_states = np.asarray(hidden_states, dtype=np.float32)
    position_ids = np.asarray(position_ids)
    Wqkv = np.asarray(Wqkv, dtype=np.float32)
    Wout = np.asarray(Wout, dtype=np.float32)

    if "nc" not in _cached:
        _cached["nc"] = _build_program()
    nc = _cached["nc"]

    inv_freq = (1.0 / (THETA ** (np.arange(0, RD, 2, dtype=np.float64) / RD))
                ).astype(np.float32)
    ident = np.eye(128, dtype=np.float32)
    rr = np.arange(128)[:, None]
    ccol = np.arange(512)[None, :]
    msk = np.stack([np.where(ccol <= 128 * p + rr, 0.0, NEG)
                    for p in range(4)], axis=1).astype(np.float32)  # [128,4,512]

    in_maps = []
    for c in range(8):
        b, r = c // 4, c % 4
        heads = list(range(4 * r, 4 * r + 4))
        hiddenT = np.ascontiguousarray(
            hidden_states[b].T.reshape(HID, 32, 128).transpose(1, 0, 2))
        rows = []
        for sec in range(3):  # q, k, v sections of Wqkv
            for h in heads:
                rows.append(Wqkv[sec * HID + h * HD:sec * HID + (h + 1) * HD])
        wqkvT = np.ascontiguousarray(np.concatenate(rows, axis=0).T)
        hperm = np.array([(4 * cc + j) * HD + d
                          for j in range(4) for cc in range(4)
                          for d in range(HD)])
        woutTp = np.ascontiguousarray(Wout[r * 1024:(r + 1) * 1024][:, hperm].T)
        pos = position_ids[b].astype(np.float32)
        fr = pos[:, None] * inv_freq[None, :]
        in_maps.append({
            "hiddenT": hiddenT, "wqkvT": wqkvT, "woutTp": woutTp,
            "cs": np.cos(fr).astype(np.float32),
            "sn": np.sin(fr).astype(np.float32),
            "msk": msk, "ident": ident,
        })

    res = run_bass_kernel_spmd(nc, in_maps, list(range(8))).results
    out = np.empty((B, S, HID), dtype=np.float32)
    for b in range(B):
        for r in range(4):
            out[b][:, r * 1024:(r + 1) * 1024] = res[4 * b + r]["out"]
    return out



# revision 3
# speedup vs baseline: 4.3077x; 4.3077x over previous
import sys
import time
import numpy as np

sys.path.insert(0, '/opt/trn_rl_repo')

import concourse.bass as bass
import concourse.bacc as bacc
import concourse.tile as tile
from concourse import mybir
from concourse.bass_utils import run_bass_kernel_spmd
from contextlib import ExitStack

F32 = mybir.dt.float32
F16 = mybir.dt.float16

B, S, HID = 2, 4096, 4096
NH, HD = 16, 256
RD = 64
THETA = 10000.0
T = B * S            # 8192 flat tokens
TPC = T // 8         # 1024 tokens per core
NEG = -30000.0

_cached = {}


def _build_program():
    nc = bacc.Bacc("TRN2", target_bir_lowering=False, debug=False, num_devices=8)
    # per-core inputs, all fp16 on the wire:
    #   hid:  this core's 1024-token slice of flattened hidden [T, HID]
    #   wqkv: rows [q(h0) q(h1) k(h0) k(h1) v(h0) v(h1)] x 256 for its 2 heads
    #   woutN: Wout[:, 512c:512c+512] (natural layout, transposed on device)
    hid_e = nc.declare_dram_parameter("hid", [TPC, HID], F16, isOutput=False)
    wqkv_e = nc.declare_dram_parameter("wqkv", [1536, HID], F16, isOutput=False)
    wout_e = nc.declare_dram_parameter("woutN", [HID, 512], F16, isOutput=False)
    cs_e = nc.declare_dram_parameter("cs", [64, 128, 32], F16, isOutput=False)
    sn_e = nc.declare_dram_parameter("sn", [64, 128, 32], F16, isOutput=False)
    msk_e = nc.declare_dram_parameter("msk", [128, 2048], F16, isOutput=False)
    id_e = nc.declare_dram_parameter("ident", [128, 128], F16, isOutput=False)
    out_e = nc.declare_dram_parameter("out", [TPC, HID], F16, isOutput=True)

    Copy = mybir.ActivationFunctionType.Copy
    Exp = mybir.ActivationFunctionType.Exp
    AX = mybir.AxisListType.X

    with tile.TileContext(nc) as tc:
        with tc.tile_pool(name="dram", bufs=1, space="DRAM") as dram, \
             tc.tile_pool(name="consts", bufs=1) as consts:
            hTs = dram.tile([HID, TPC], F16)       # hidden^T, my token slice
            gt = dram.tile([8, HID, TPC], F16)     # allgathered hidden^T
            QT = dram.tile([512, T], F16)          # q^T for my 2 heads (rope'd)
            KT = dram.tile([512, T], F16)
            VN = dram.tile([T, 512], F16)          # v, natural [token, d]
            AT = dram.tile([512, T], F16)          # attn out^T for my 2 heads
            PO = dram.tile([T, HID], F16)          # partial out-proj
            RSo = dram.tile([TPC, HID], F16)       # reduce-scattered slice

            idt = consts.tile([128, 128], F16, name="idt", tag="idt")
            nc.sync.dma_start(out=idt, in_=id_e.ap())
            csf = consts.tile([128, 64, 32], F32, name="csf", tag="csf")
            snf = consts.tile([128, 64, 32], F32, name="snf", tag="snf")
            mskf = consts.tile([128, 2048], F32, name="mskf", tag="mskf")

            # ---------- phase 0: transpose own hidden slice, allgather ------
            with ExitStack() as s0:
                hin = s0.enter_context(tc.tile_pool(name="hin", bufs=2))
                hout = s0.enter_context(tc.tile_pool(name="hout", bufs=2))
                pst0 = s0.enter_context(tc.tile_pool(name="pst0", bufs=4, space="PSUM"))
                hTv = hTs.rearrange("(kc p) t -> p kc t", p=128)
                for tt in range(8):
                    hs = hin.tile([128, HID], F16, name="hs")
                    nc.sync.dma_start(out=hs, in_=hid_e.ap()[tt * 128:(tt + 1) * 128, :])
                    hb = hout.tile([128, 32, 128], F16, name="hb")
                    for kc in range(32):
                        tp = pst0.tile([128, 128], F16, name="tp0")
                        nc.tensor.transpose(tp, hs[:, kc * 128:(kc + 1) * 128], idt)
                        nc.vector.tensor_copy(hb[:, kc, :], tp)
                    nc.sync.dma_start(out=hTv[:, :, tt * 128:(tt + 1) * 128], in_=hb)
                nc.gpsimd.collective_compute(
                    "AllGather", mybir.AluOpType.bypass,
                    replica_groups=[list(range(8))],
                    ins=[hTs[:]], outs=[gt[:]])

            # ---------- phase 1: QKV projection + RoPE + transposes ---------
            with ExitStack() as s1:
                wq = s1.enter_context(tc.tile_pool(name="wq", bufs=1))
                wn = s1.enter_context(tc.tile_pool(name="wn", bufs=2))
                hstr = s1.enter_context(tc.tile_pool(name="hstr", bufs=2))
                ev = s1.enter_context(tc.tile_pool(name="ev", bufs=4))
                tr = s1.enter_context(tc.tile_pool(name="tr", bufs=4))
                pmm = s1.enter_context(tc.tile_pool(name="pmm", bufs=2, space="PSUM"))
                ptr = s1.enter_context(tc.tile_pool(name="ptr", bufs=4, space="PSUM"))

                # load + upcast cos/sin/mask constants
                cst = ev.tile([128, 64, 32], F16, name="cst", bufs=1)
                nc.sync.dma_start(out=cst, in_=cs_e.ap().rearrange("tt p f -> p tt f"))
                nc.scalar.activation(csf.rearrange("p a b -> p (a b)"),
                                     cst.rearrange("p a b -> p (a b)"), Copy)
                snt = ev.tile([128, 64, 32], F16, name="snt", bufs=1)
                nc.sync.dma_start(out=snt, in_=sn_e.ap().rearrange("tt p f -> p tt f"))
                nc.scalar.activation(snf.rearrange("p a b -> p (a b)"),
                                     snt.rearrange("p a b -> p (a b)"), Copy)
                mskst = ev.tile([128, 2048], F16, name="mskst", bufs=1)
                nc.sync.dma_start(out=mskst, in_=msk_e.ap())
                nc.scalar.activation(mskf, mskst, Copy)

                # device-side transpose of wqkv -> 32 resident [128k, 1536o]
                wqkvT = [wq.tile([128, 1536], F16, name=f"wt{kc}", tag=f"wt{kc}")
                         for kc in range(32)]
                for j in range(12):
                    wnat = wn.tile([128, HID], F16, name="wnat")
                    nc.sync.dma_start(out=wnat,
                                      in_=wqkv_e.ap()[j * 128:(j + 1) * 128, :])
                    for kc in range(32):
                        tp = ptr.tile([128, 128], F16, name="tp1")
                        nc.tensor.transpose(tp, wnat[:, kc * 128:(kc + 1) * 128], idt)
                        nc.vector.tensor_copy(wqkvT[kc][:, j * 128:(j + 1) * 128], tp)

                gv = gt.rearrange("blk (kc p) t -> blk p kc t", p=128)
                for tt in range(64):
                    blk, ts = tt // 8, (tt % 8) * 128
                    hT = hstr.tile([128, 32, 128], F16, name="hT")
                    nc.sync.dma_start(out=hT, in_=gv[blk, :, :, ts:ts + 128])
                    for oc in range(3):
                        ps = pmm.tile([128, 512], F32, name="qkvps")
                        for kc in range(32):
                            nc.tensor.matmul(
                                ps, hT[:, kc, :],
                                wqkvT[kc][:, oc * 512:(oc + 1) * 512],
                                start=(kc == 0), stop=(kc == 31))
                        ot = ev.tile([128, 512], F16, name="ot")
                        if oc < 2:
                            # GPT-J interleaved rope on first 64 dims per head;
                            # rotated pairs written deinterleaved (blocks of 32)
                            # -- ok since q and k get the same permutation.
                            for h in range(2):
                                b0 = h * 256
                                x1 = ps[:, b0 + 0:b0 + 64:2]
                                x2 = ps[:, b0 + 1:b0 + 65:2]
                                ct = csf[:, tt, :]
                                st_ = snf[:, tt, :]
                                ta = tr.tile([128, 32], F32, name="ta")
                                tb = tr.tile([128, 32], F32, name="tb")
                                nc.vector.tensor_mul(ta, x1, ct)
                                nc.vector.tensor_mul(tb, x2, st_)
                                nc.vector.tensor_sub(ot[:, b0:b0 + 32], ta, tb)
                                tc2 = tr.tile([128, 32], F32, name="tc2")
                                td = tr.tile([128, 32], F32, name="td")
                                nc.vector.tensor_mul(tc2, x2, ct)
                                nc.vector.tensor_mul(td, x1, st_)
                                nc.vector.tensor_add(ot[:, b0 + 32:b0 + 64], tc2, td)
                                nc.scalar.activation(ot[:, b0 + 64:b0 + 256],
                                                     ps[:, b0 + 64:b0 + 256], Copy)
                            dst = QT if oc == 0 else KT
                            for db in range(4):
                                tp = ptr.tile([128, 128], F16, name="tp1")
                                nc.tensor.transpose(tp, ot[:, db * 128:(db + 1) * 128], idt)
                                ob = ev.tile([128, 128], F16, name="ob")
                                nc.vector.tensor_copy(ob, tp)
                                nc.sync.dma_start(
                                    out=dst[db * 128:(db + 1) * 128,
                                            tt * 128:(tt + 1) * 128],
                                    in_=ob)
                        else:
                            nc.scalar.activation(ot, ps, Copy)
                            nc.sync.dma_start(
                                out=VN[tt * 128:(tt + 1) * 128, :], in_=ot)

            # ---------- phase 2: causal attention for my 2 heads ------------
            with ExitStack() as s2:
                kvp = s2.enter_context(tc.tile_pool(name="kvp", bufs=2))
                pts = s2.enter_context(tc.tile_pool(name="pts", bufs=1))
                sp = s2.enter_context(tc.tile_pool(name="sp", bufs=2))
                sm = s2.enter_context(tc.tile_pool(name="sm", bufs=4))
                aot = s2.enter_context(tc.tile_pool(name="aot", bufs=3))
                pss = s2.enter_context(tc.tile_pool(name="pss", bufs=2, space="PSUM"))
                pso = s2.enter_context(tc.tile_pool(name="pso", bufs=1, space="PSUM"))
                ptp = s2.enter_context(tc.tile_pool(name="ptp", bufs=4, space="PSUM"))
                vv = VN.rearrange("(g p) d -> p g d", p=128)
                for h in range(2):
                    for b in range(2):
                        q2, k2 = [], []
                        for d in range(2):
                            qt_ = kvp.tile([128, S], F16, name=f"qt{d}")
                            nc.sync.dma_start(
                                out=qt_,
                                in_=QT[h * 256 + d * 128:h * 256 + (d + 1) * 128,
                                       b * S:(b + 1) * S])
                            q2.append(qt_)
                            kt_ = kvp.tile([128, S], F16, name=f"kt{d}")
                            nc.sync.dma_start(
                                out=kt_,
                                in_=KT[h * 256 + d * 128:h * 256 + (d + 1) * 128,
                                       b * S:(b + 1) * S])
                            k2.append(kt_)
                        vt = kvp.tile([128, 32, 256], F16, name="vt", bufs=1)
                        nc.sync.dma_start(
                            out=vt, in_=vv[:, b * 32:(b + 1) * 32,
                                           h * 256:(h + 1) * 256])
                        for qb in range(8):
                            nk = qb + 1
                            pt_t = pts.tile([128, 32, 512], F16, name="ptt")
                            for qs in range(4):
                                qo = qb * 512 + qs * 128
                                prow = sp.tile([128, 4096], F16, name="prow")
                                sums = sm.tile([128, 8], F32, name="sums")
                                for kc in range(nk):
                                    ps_ = pss.tile([128, 512], F32, name="sps")
                                    for d in range(2):
                                        nc.tensor.matmul(
                                            ps_, q2[d][:, qo:qo + 128],
                                            k2[d][:, kc * 512:(kc + 1) * 512],
                                            start=(d == 0), stop=(d == 1))
                                    if kc == qb:
                                        srow = sm.tile([128, 512], F32, name="srow")
                                        nc.vector.tensor_add(
                                            srow, ps_,
                                            mskf[:, qs * 512:(qs + 1) * 512])
                                        nc.scalar.activation(
                                            prow[:, kc * 512:(kc + 1) * 512],
                                            srow, Exp, scale=1.0 / 16.0,
                                            accum_out=sums[:, kc:kc + 1])
                                    else:
                                        nc.scalar.activation(
                                            prow[:, kc * 512:(kc + 1) * 512],
                                            ps_, Exp, scale=1.0 / 16.0,
                                            accum_out=sums[:, kc:kc + 1])
                                ssum = sm.tile([128, 1], F32, name="ssum")
                                nc.vector.reduce_sum(ssum, sums[:, 0:nk], axis=AX)
                                rinv = sm.tile([128, 1], F32, name="rinv")
                                nc.vector.reciprocal(rinv, ssum)
                                pscl = sp.tile([128, 4096], F16, name="pscl")
                                nc.vector.tensor_scalar_mul(
                                    pscl[:, 0:nk * 512], prow[:, 0:nk * 512], rinv)
                                for g in range(nk * 4):
                                    tp = ptp.tile([128, 128], F16, name="ptp")
                                    nc.tensor.transpose(
                                        tp, pscl[:, g * 128:(g + 1) * 128], idt)
                                    nc.vector.tensor_copy(
                                        pt_t[:, g, qs * 128:(qs + 1) * 128], tp)
                            po2 = [pso.tile([128, 512], F32, name=f"po{d}")
                                   for d in range(2)]
                            for g in range(nk * 4):
                                for d in range(2):
                                    nc.tensor.matmul(
                                        po2[d], vt[:, g, d * 128:(d + 1) * 128],
                                        pt_t[:, g, :],
                                        start=(g == 0), stop=(g == nk * 4 - 1))
                            for d in range(2):
                                ao = aot.tile([128, 512], F16, name="ao")
                                nc.scalar.activation(ao, po2[d], Copy)
                                nc.sync.dma_start(
                                    out=AT[h * 256 + d * 128:h * 256 + (d + 1) * 128,
                                           b * S + qb * 512:b * S + (qb + 1) * 512],
                                    in_=ao)

            # ---------- phase 3: output projection + reduce-scatter ---------
            with ExitStack() as s3:
                wo4 = s3.enter_context(tc.tile_pool(name="wo4", bufs=1))
                wos = s3.enter_context(tc.tile_pool(name="wos", bufs=2))
                ap_ = s3.enter_context(tc.tile_pool(name="ap", bufs=2))
                ob_ = s3.enter_context(tc.tile_pool(name="obp", bufs=3))
                pf = s3.enter_context(tc.tile_pool(name="pf", bufs=2, space="PSUM"))
                ptw = s3.enter_context(tc.tile_pool(name="ptw", bufs=4, space="PSUM"))
                w4 = wo4.tile([128, 4, HID], F16, name="w4", tag="w4")
                for j in range(32):
                    wns = wos.tile([128, 512], F16, name="wns")
                    nc.sync.dma_start(out=wns,
                                      in_=wout_e.ap()[j * 128:(j + 1) * 128, :])
                    for dc in range(4):
                        tp = ptw.tile([128, 128], F16, name="wtp2")
                        nc.tensor.transpose(tp, wns[:, dc * 128:(dc + 1) * 128], idt)
                        nc.vector.tensor_copy(w4[:, dc, j * 128:(j + 1) * 128], tp)
                atv = AT.rearrange("(dc p) t -> p dc t", p=128)
                for tt in range(64):
                    at = ap_.tile([128, 4, 128], F16, name="at")
                    nc.sync.dma_start(out=at, in_=atv[:, :, tt * 128:(tt + 1) * 128])
                    oto = ob_.tile([128, HID], F16, name="oto")
                    for oc in range(8):
                        ps2 = pf.tile([128, 512], F32, name="ps2")
                        for dc in range(4):
                            nc.tensor.matmul(
                                ps2, at[:, dc, :],
                                w4[:, dc, oc * 512:(oc + 1) * 512],
                                start=(dc == 0), stop=(dc == 3))
                        nc.scalar.activation(oto[:, oc * 512:(oc + 1) * 512], ps2, Copy)
                    nc.sync.dma_start(out=PO[tt * 128:(tt + 1) * 128, :], in_=oto)
                nc.gpsimd.collective_compute(
                    "ReduceScatter", mybir.AluOpType.add,
                    replica_groups=[list(range(8))],
                    ins=[PO[:]], outs=[RSo[:]])
                for i in range(8):
                    t_ = ob_.tile([128, HID], F16, name="cpy", bufs=2)
                    nc.sync.dma_start(out=t_, in_=RSo[i * 128:(i + 1) * 128, :])
                    nc.sync.dma_start(out=out_e.ap()[i * 128:(i + 1) * 128, :], in_=t_)

    nc.compile()
    return nc


def kernel(hidden_states, position_ids, Wqkv, Wout):
    t0 = time.time()
    hs = np.asarray(hidden_states, dtype=np.float32).reshape(T, HID)
    pos = np.asarray(position_ids).reshape(T).astype(np.float32)
    Wqkv = np.asarray(Wqkv, dtype=np.float32)
    Wout = np.asarray(Wout, dtype=np.float32)

    if "nc" not in _cached:
        _cached["nc"] = _build_program()
    nc = _cached["nc"]
    t1 = time.time()

    inv_freq = (1.0 / (THETA ** (np.arange(0, RD, 2, dtype=np.float64) / RD))
                ).astype(np.float32)
    fr = pos[:, None] * inv_freq[None, :]
    cs = np.cos(fr).astype(np.float16).reshape(64, 128, 32)
    sn = np.sin(fr).astype(np.float16).reshape(64, 128, 32)
    rr = np.arange(128)[:, None]
    cc = np.arange(512)[None, :]
    msk = np.concatenate([np.where(cc <= 128 * q + rr, 0.0, NEG)
                          for q in range(4)], axis=1).astype(np.float16)
    ident = np.eye(128, dtype=np.float16)
    h16 = hs.astype(np.float16)
    wq16 = Wqkv.astype(np.float16).reshape(3, 8, 512, HID)
    wo16 = Wout.astype(np.float16)

    in_maps = []
    for c in range(8):
        in_maps.append({
            "hid": h16[c * TPC:(c + 1) * TPC],
            "wqkv": np.ascontiguousarray(wq16[:, c]).reshape(1536, HID),
            "woutN": np.ascontiguousarray(wo16[:, c * 512:(c + 1) * 512]),
            "cs": cs, "sn": sn, "msk": msk, "ident": ident,
        })
    t2 = time.time()

    res = run_bass_kernel_spmd(nc, in_maps, list(range(8))).results
    t3 = time.time()

    out = np.concatenate([res[c]["out"] for c in range(8)], axis=0)
    out = out.astype(np.float32).reshape(B, S, HID)
    t4 = time.time()
    print(f"[kernel] build={t1 - t0:.2f}s prep={t2 - t1:.2f}s "
          f"run={t3 - t2:.2f}s post={t4 - t3:.2f}s", file=sys.stderr)
    return out


# revision 5
# speedup vs baseline: 4.7247x; 1.0968x over previous
import sys
import time
import numpy as np

sys.path.insert(0, '/opt/trn_rl_repo')

import concourse.bass as bass
import concourse.bacc as bacc
import concourse.tile as tile
from concourse import mybir
from concourse.bass_utils import run_bass_kernel_spmd
from contextlib import ExitStack

F32 = mybir.dt.float32
F16 = mybir.dt.float16

B, S, HID = 2, 4096, 4096
NH, HD = 16, 256
RD = 64
THETA = 10000.0
T = B * S            # 8192 flat tokens
TPC = T // 8         # 1024 tokens per core
NEG = -30000.0

_cached = {}


def _build_program():
    nc = bacc.Bacc("TRN2", target_bir_lowering=False, debug=False, num_devices=8)
    # per-core inputs, all fp16 on the wire:
    #   hid:  this core's 1024-token slice of flattened hidden [T, HID]
    #   wqkv: rows [q(h0) q(h1) k(h0) k(h1) v(h0) v(h1)] x 256 for its 2 heads
    #   woutN: Wout[:, 512c:512c+512] (natural layout, transposed on device)
    hid_e = nc.declare_dram_parameter("hid", [TPC, HID], F16, isOutput=False)
    wqkv_e = nc.declare_dram_parameter("wqkv", [1536, HID], F16, isOutput=False)
    wout_e = nc.declare_dram_parameter("woutN", [HID, 512], F16, isOutput=False)
    cs_e = nc.declare_dram_parameter("cs", [64, 128, 32], F16, isOutput=False)
    sn_e = nc.declare_dram_parameter("sn", [64, 128, 32], F16, isOutput=False)
    msk_e = nc.declare_dram_parameter("msk", [128, 2048], F16, isOutput=False)
    id_e = nc.declare_dram_parameter("ident", [128, 128], F16, isOutput=False)
    out_e = nc.declare_dram_parameter("out", [TPC, HID], F16, isOutput=True)

    Copy = mybir.ActivationFunctionType.Copy
    Exp = mybir.ActivationFunctionType.Exp
    AX = mybir.AxisListType.X

    with tile.TileContext(nc) as tc:
        with tc.tile_pool(name="dram", bufs=1, space="DRAM") as dram, \
             tc.tile_pool(name="consts", bufs=1) as consts:
            hTs = dram.tile([HID, TPC], F16)       # hidden^T, my token slice
            gt = dram.tile([8, HID, TPC], F16)     # allgathered hidden^T
            QT = dram.tile([512, T], F16)          # q^T for my 2 heads (rope'd)
            KT = dram.tile([512, T], F16)
            VN = dram.tile([T, 512], F16)          # v, natural [token, d]
            AT = dram.tile([512, T], F16)          # attn out^T for my 2 heads
            PO = dram.tile([T, HID], F16)          # partial out-proj
            RSo = dram.tile([TPC, HID], F16)       # reduce-scattered slice

            idt = consts.tile([128, 128], F16, name="idt", tag="idt")
            nc.sync.dma_start(out=idt, in_=id_e.ap())
            csf = consts.tile([128, 64, 32], F32, name="csf", tag="csf")
            snf = consts.tile([128, 64, 32], F32, name="snf", tag="snf")
            mskf = consts.tile([128, 2048], F32, name="mskf", tag="mskf")

            # ---------- phase 0: transpose own hidden slice, allgather ------
            with ExitStack() as s0:
                hin = s0.enter_context(tc.tile_pool(name="hin", bufs=2))
                hout = s0.enter_context(tc.tile_pool(name="hout", bufs=2))
                pst0 = s0.enter_context(tc.tile_pool(name="pst0", bufs=4, space="PSUM"))
                hTv = hTs.rearrange("(kc p) t -> p kc t", p=128)
                for tt in range(8):
                    hs = hin.tile([128, HID], F16, name="hs")
                    nc.sync.dma_start(out=hs, in_=hid_e.ap()[tt * 128:(tt + 1) * 128, :])
                    hb = hout.tile([128, 32, 128], F16, name="hb")
                    for kc in range(32):
                        tp = pst0.tile([128, 128], F16, name="tp0")
                        nc.tensor.transpose(tp, hs[:, kc * 128:(kc + 1) * 128], idt)
                        nc.vector.tensor_copy(hb[:, kc, :], tp)
                    nc.sync.dma_start(out=hTv[:, :, tt * 128:(tt + 1) * 128], in_=hb)
                nc.gpsimd.collective_compute(
                    "AllGather", mybir.AluOpType.bypass,
                    replica_groups=[list(range(8))],
                    ins=[hTs[:]], outs=[gt[:]])

            # ---------- phase 1: QKV projection + RoPE + transposes ---------
            with ExitStack() as s1:
                wq = s1.enter_context(tc.tile_pool(name="wq", bufs=1))
                wn = s1.enter_context(tc.tile_pool(name="wn", bufs=2))
                hstr = s1.enter_context(tc.tile_pool(name="hstr", bufs=2))
                ev = s1.enter_context(tc.tile_pool(name="ev", bufs=4))
                tr = s1.enter_context(tc.tile_pool(name="tr", bufs=4))
                pmm = s1.enter_context(tc.tile_pool(name="pmm", bufs=2, space="PSUM"))
                ptr = s1.enter_context(tc.tile_pool(name="ptr", bufs=4, space="PSUM"))

                # load + upcast cos/sin/mask constants
                cst = ev.tile([128, 64, 32], F16, name="cst", bufs=1)
                nc.sync.dma_start(out=cst, in_=cs_e.ap().rearrange("tt p f -> p tt f"))
                nc.scalar.activation(csf.rearrange("p a b -> p (a b)"),
                                     cst.rearrange("p a b -> p (a b)"), Copy)
                snt = ev.tile([128, 64, 32], F16, name="snt", bufs=1)
                nc.sync.dma_start(out=snt, in_=sn_e.ap().rearrange("tt p f -> p tt f"))
                nc.scalar.activation(snf.rearrange("p a b -> p (a b)"),
                                     snt.rearrange("p a b -> p (a b)"), Copy)
                mskst = ev.tile([128, 2048], F16, name="mskst", bufs=1)
                nc.sync.dma_start(out=mskst, in_=msk_e.ap())
                nc.scalar.activation(mskf, mskst, Copy)

                # device-side transpose of wqkv -> 32 resident [128k, 1536o]
                wqkvT = [wq.tile([128, 1536], F16, name=f"wt{kc}", tag=f"wt{kc}")
                         for kc in range(32)]
                for j in range(12):
                    wnat = wn.tile([128, HID], F16, name="wnat")
                    nc.sync.dma_start(out=wnat,
                                      in_=wqkv_e.ap()[j * 128:(j + 1) * 128, :])
                    for kc in range(32):
                        tp = ptr.tile([128, 128], F16, name="tp1")
                        nc.tensor.transpose(tp, wnat[:, kc * 128:(kc + 1) * 128], idt)
                        nc.vector.tensor_copy(wqkvT[kc][:, j * 128:(j + 1) * 128], tp)

                gv = gt.rearrange("blk (kc p) t -> blk p kc t", p=128)
                for tt in range(64):
                    blk, ts = tt // 8, (tt % 8) * 128
                    hT = hstr.tile([128, 32, 128], F16, name="hT")
                    nc.sync.dma_start(out=hT, in_=gv[blk, :, :, ts:ts + 128])
                    for oc in range(3):
                        ps = pmm.tile([128, 512], F32, name="qkvps")
                        for kc in range(32):
                            nc.tensor.matmul(
                                ps, hT[:, kc, :],
                                wqkvT[kc][:, oc * 512:(oc + 1) * 512],
                                start=(kc == 0), stop=(kc == 31))
                        ot = ev.tile([128, 512], F16, name="ot")
                        if oc < 2:
                            # GPT-J interleaved rope on first 64 dims per head;
                            # rotated pairs written deinterleaved (blocks of 32)
                            # -- ok since q and k get the same permutation.
                            for h in range(2):
                                b0 = h * 256
                                x1 = ps[:, b0 + 0:b0 + 64:2]
                                x2 = ps[:, b0 + 1:b0 + 65:2]
                                ct = csf[:, tt, :]
                                st_ = snf[:, tt, :]
                                ta = tr.tile([128, 32], F32, name="ta")
                                tb = tr.tile([128, 32], F32, name="tb")
                                nc.vector.tensor_mul(ta, x1, ct)
                                nc.vector.tensor_mul(tb, x2, st_)
                                nc.vector.tensor_sub(ot[:, b0:b0 + 32], ta, tb)
                                tc2 = tr.tile([128, 32], F32, name="tc2")
                                td = tr.tile([128, 32], F32, name="td")
                                nc.vector.tensor_mul(tc2, x2, ct)
                                nc.vector.tensor_mul(td, x1, st_)
                                nc.vector.tensor_add(ot[:, b0 + 32:b0 + 64], tc2, td)
                                nc.scalar.activation(ot[:, b0 + 64:b0 + 256],
                                                     ps[:, b0 + 64:b0 + 256], Copy)
                            dst = QT if oc == 0 else KT
                            for db in range(4):
                                tp = ptr.tile([128, 128], F16, name="tp1")
                                nc.tensor.transpose(tp, ot[:, db * 128:(db + 1) * 128], idt)
                                ob = ev.tile([128, 128], F16, name="ob")
                                nc.vector.tensor_copy(ob, tp)
                                nc.sync.dma_start(
                                    out=dst[db * 128:(db + 1) * 128,
                                            tt * 128:(tt + 1) * 128],
                                    in_=ob)
                        else:
                            nc.scalar.activation(ot, ps, Copy)
                            nc.sync.dma_start(
                                out=VN[tt * 128:(tt + 1) * 128, :], in_=ot)

            # ---------- phase 2: causal attention for my 2 heads ------------
            with ExitStack() as s2:
                kvp = s2.enter_context(tc.tile_pool(name="kvp", bufs=2))
                pts = s2.enter_context(tc.tile_pool(name="pts", bufs=1))
                sp = s2.enter_context(tc.tile_pool(name="sp", bufs=2))
                sm = s2.enter_context(tc.tile_pool(name="sm", bufs=4))
                aot = s2.enter_context(tc.tile_pool(name="aot", bufs=3))
                pss = s2.enter_context(tc.tile_pool(name="pss", bufs=2, space="PSUM"))
                pso = s2.enter_context(tc.tile_pool(name="pso", bufs=1, space="PSUM"))
                ptp = s2.enter_context(tc.tile_pool(name="ptp", bufs=4, space="PSUM"))
                vv = VN.rearrange("(g p) d -> p g d", p=128)
                for h in range(2):
                    for b in range(2):
                        q2, k2 = [], []
                        for d in range(2):
                            qt_ = kvp.tile([128, S], F16, name=f"qt{d}")
                            nc.sync.dma_start(
                                out=qt_,
                                in_=QT[h * 256 + d * 128:h * 256 + (d + 1) * 128,
                                       b * S:(b + 1) * S])
                            q2.append(qt_)
                            kt_ = kvp.tile([128, S], F16, name=f"kt{d}")
                            nc.sync.dma_start(
                                out=kt_,
                                in_=KT[h * 256 + d * 128:h * 256 + (d + 1) * 128,
                                       b * S:(b + 1) * S])
                            k2.append(kt_)
                        vt = kvp.tile([128, 32, 256], F16, name="vt", bufs=1)
                        nc.sync.dma_start(
                            out=vt, in_=vv[:, b * 32:(b + 1) * 32,
                                           h * 256:(h + 1) * 256])
                        for qb in range(8):
                            nk = qb + 1
                            pt_t = pts.tile([128, 32, 512], F16, name="ptt")
                            for qs in range(4):
                                qo = qb * 512 + qs * 128
                                prow = sp.tile([128, 4096], F16, name="prow")
                                sums = sm.tile([128, 8], F32, name="sums")
                                for kc in range(nk):
                                    ps_ = pss.tile([128, 512], F32, name="sps")
                                    for d in range(2):
                                        nc.tensor.matmul(
                                            ps_, q2[d][:, qo:qo + 128],
                                            k2[d][:, kc * 512:(kc + 1) * 512],
                                            start=(d == 0), stop=(d == 1))
                                    if kc == qb:
                                        srow = sm.tile([128, 512], F32, name="srow")
                                        nc.vector.tensor_add(
                                            srow, ps_,
                                            mskf[:, qs * 512:(qs + 1) * 512])
                                        nc.scalar.activation(
                                            prow[:, kc * 512:(kc + 1) * 512],
                                            srow, Exp, scale=1.0 / 16.0,
                                            accum_out=sums[:, kc:kc + 1])
                                    else:
                                        nc.scalar.activation(
                                            prow[:, kc * 512:(kc + 1) * 512],
                                            ps_, Exp, scale=1.0 / 16.0,
                                            accum_out=sums[:, kc:kc + 1])
                                ssum = sm.tile([128, 1], F32, name="ssum")
                                nc.vector.reduce_sum(ssum, sums[:, 0:nk], axis=AX)
                                rinv = sm.tile([128, 1], F32, name="rinv")
                                nc.vector.reciprocal(rinv, ssum)
                                pscl = sp.tile([128, 4096], F16, name="pscl")
                                nc.vector.tensor_scalar_mul(
                                    pscl[:, 0:nk * 512], prow[:, 0:nk * 512], rinv)
                                for g in range(nk * 4):
                                    tp = ptp.tile([128, 128], F16, name="ptp")
                                    nc.tensor.transpose(
                                        tp, pscl[:, g * 128:(g + 1) * 128], idt)
                                    nc.vector.tensor_copy(
                                        pt_t[:, g, qs * 128:(qs + 1) * 128], tp)
                            po2 = [pso.tile([128, 512], F32, name=f"po{d}")
                                   for d in range(2)]
                            for g in range(nk * 4):
                                for d in range(2):
                                    nc.tensor.matmul(
                                        po2[d], vt[:, g, d * 128:(d + 1) * 128],
                                        pt_t[:, g, :],
                                        start=(g == 0), stop=(g == nk * 4 - 1))
                            for d in range(2):
                                ao = aot.tile([128, 512], F16, name="ao")
                                nc.scalar.activation(ao, po2[d], Copy)
                                nc.sync.dma_start(
                                    out=AT[h * 256 + d * 128:h * 256 + (d + 1) * 128,
                                           b * S + qb * 512:b * S + (qb + 1) * 512],
                                    in_=ao)

            # ---------- phase 3: output projection + reduce-scatter ---------
            with ExitStack() as s3:
                wo4 = s3.enter_context(tc.tile_pool(name="wo4", bufs=1))
                wos = s3.enter_context(tc.tile_pool(name="wos", bufs=2))
                ap_ = s3.enter_context(tc.tile_pool(name="ap", bufs=2))
                ob_ = s3.enter_context(tc.tile_pool(name="obp", bufs=3))
                pf = s3.enter_context(tc.tile_pool(name="pf", bufs=2, space="PSUM"))
                ptw = s3.enter_context(tc.tile_pool(name="ptw", bufs=4, space="PSUM"))
                w4 = wo4.tile([128, 4, HID], F16, name="w4", tag="w4")
                for j in range(32):
                    wns = wos.tile([128, 512], F16, name="wns")
                    nc.sync.dma_start(out=wns,
                                      in_=wout_e.ap()[j * 128:(j + 1) * 128, :])
                    for dc in range(4):
                        tp = ptw.tile([128, 128], F16, name="wtp2")
                        nc.tensor.transpose(tp, wns[:, dc * 128:(dc + 1) * 128], idt)
                        nc.vector.tensor_copy(w4[:, dc, j * 128:(j + 1) * 128], tp)
                atv = AT.rearrange("(dc p) t -> p dc t", p=128)
                for tt in range(64):
                    at = ap_.tile([128, 4, 128], F16, name="at")
                    nc.sync.dma_start(out=at, in_=atv[:, :, tt * 128:(tt + 1) * 128])
                    oto = ob_.tile([128, HID], F16, name="oto")
                    for oc in range(8):
                        ps2 = pf.tile([128, 512], F32, name="ps2")
                        for dc in range(4):
                            nc.tensor.matmul(
                                ps2, at[:, dc, :],
                                w4[:, dc, oc * 512:(oc + 1) * 512],
                                start=(dc == 0), stop=(dc == 3))
                        nc.scalar.activation(oto[:, oc * 512:(oc + 1) * 512], ps2, Copy)
                    nc.sync.dma_start(out=PO[tt * 128:(tt + 1) * 128, :], in_=oto)
                nc.gpsimd.collective_compute(
                    "ReduceScatter", mybir.AluOpType.add,
                    replica_groups=[list(range(8))],
                    ins=[PO[:]], outs=[RSo[:]])
                for i in range(8):
                    t_ = ob_.tile([128, HID], F16, name="cpy", bufs=2)
                    nc.sync.dma_start(out=t_, in_=RSo[i * 128:(i + 1) * 128, :])
                    nc.sync.dma_start(out=out_e.ap()[i * 128:(i + 1) * 128, :], in_=t_)

    nc.compile()
    return nc


def _make_runner(nc):
    """Build a cached jitted executor for nc (trace/lower once, reuse)."""
    import jax
    import jax.numpy as jnp
    from jax.sharding import Mesh, PartitionSpec, NamedSharding
    try:
        from jax.experimental.shard_map import shard_map
    except ImportError:
        from jax import shard_map
    from concourse import bass2jax as b2j

    b2j.install_neuronx_cc_hook()
    assert nc.dbg_addr is None
    partition_name = nc.partition_id_tensor.name if nc.partition_id_tensor else None
    in_names, out_names, out_avals = [], [], []
    for alloc in nc.m.functions[0].allocations:
        if not isinstance(alloc, mybir.MemoryLocationSet):
            continue
        name = alloc.memorylocations[0].name
        if alloc.kind == "ExternalInput":
            if name != partition_name:
                in_names.append(name)
        elif alloc.kind == "ExternalOutput":
            out_names.append(name)
            shape = tuple(alloc.tensor_shape)
            dtype = mybir.dt.np(alloc.dtype)
            out_avals.append(jax.core.ShapedArray(shape, dtype))
    n_params = len(in_names)
    all_names = tuple(in_names + out_names +
                      ([partition_name] if partition_name else []))
    donate = tuple(range(n_params, n_params + len(out_names)))

    def _body(*args):
        operands = list(args)
        if partition_name is not None:
            operands.append(b2j.partition_id_tensor())
        outs = b2j._bass_exec_p.bind(
            *operands, out_avals=tuple(out_avals), in_names=all_names,
            out_names=tuple(out_names), lowering_input_output_aliases=(),
            sim_require_finite=True, sim_require_nnan=True, nc=nc)
        return tuple(outs)

    devices = jax.devices()[:8]
    mesh = Mesh(np.asarray(devices), ("core",))
    spec = PartitionSpec("core")
    sharded = jax.jit(
        shard_map(_body, mesh=mesh,
                  in_specs=(spec,) * (n_params + len(out_names)),
                  out_specs=(spec,) * len(out_names), check_rep=False),
        donate_argnums=donate, keep_unused=True)
    sh = NamedSharding(mesh, spec)
    zero_fns = [
        jax.jit(lambda a=a: jnp.zeros((8 * a.shape[0],) + tuple(a.shape[1:]),
                                      a.dtype), out_shardings=sh)
        for a in out_avals]

    def run(in_maps):
        tA = time.time()
        gins = []
        for i, name in enumerate(in_names):
            shards = [jax.device_put(np.asarray(in_maps[c][name]), devices[c])
                      for c in range(8)]
            gshape = (8 * shards[0].shape[0],) + tuple(shards[0].shape[1:])
            gins.append(jax.make_array_from_single_device_arrays(
                gshape, sh, shards))
        zeros = [zf() for zf in zero_fns]
        for g in gins:
            g.block_until_ready()
        tB = time.time()
        outs = sharded(*gins, *zeros)
        for o in outs:
            o.block_until_ready()
        tC = time.time()
        res = {name: np.asarray(outs[i]) for i, name in enumerate(out_names)}
        tD = time.time()
        print(f"[runner] put={tB - tA:.2f}s exec={tC - tB:.2f}s "
              f"fetch={tD - tC:.2f}s", file=sys.stderr)
        return res

    return run


def kernel(hidden_states, position_ids, Wqkv, Wout):
    t0 = time.time()
    hs = np.asarray(hidden_states, dtype=np.float32).reshape(T, HID)
    pos = np.asarray(position_ids).reshape(T).astype(np.float32)
    Wqkv = np.asarray(Wqkv, dtype=np.float32)
    Wout = np.asarray(Wout, dtype=np.float32)

    if "nc" not in _cached:
        _cached["nc"] = _build_program()
    nc = _cached["nc"]
    t1 = time.time()

    inv_freq = (1.0 / (THETA ** (np.arange(0, RD, 2, dtype=np.float64) / RD))
                ).astype(np.float32)
    fr = pos[:, None] * inv_freq[None, :]
    cs = np.cos(fr).astype(np.float16).reshape(64, 128, 32)
    sn = np.sin(fr).astype(np.float16).reshape(64, 128, 32)
    rr = np.arange(128)[:, None]
    cc = np.arange(512)[None, :]
    msk = np.concatenate([np.where(cc <= 128 * q + rr, 0.0, NEG)
                          for q in range(4)], axis=1).astype(np.float16)
    ident = np.eye(128, dtype=np.float16)
    h16 = hs.astype(np.float16)
    wq16 = Wqkv.astype(np.float16).reshape(3, 8, 512, HID)
    wo16 = Wout.astype(np.float16)

    in_maps = []
    for c in range(8):
        in_maps.append({
            "hid": h16[c * TPC:(c + 1) * TPC],
            "wqkv": np.ascontiguousarray(wq16[:, c]).reshape(1536, HID),
            "woutN": np.ascontiguousarray(wo16[:, c * 512:(c + 1) * 512]),
            "cs": cs, "sn": sn, "msk": msk, "ident": ident,
        })
    t2 = time.time()

    try:
        if "runner" not in _cached:
            _cached["runner"] = _make_runner(nc)
        res8 = _cached["runner"](in_maps)
        out = res8["out"]
    except Exception as e:
        print(f"[kernel] cached runner failed ({e!r}); falling back",
              file=sys.stderr)
        _cached.pop("runner", None)
        res = run_bass_kernel_spmd(nc, in_maps, list(range(8))).results
        out = np.concatenate([res[c]["out"] for c in range(8)], axis=0)
    t3 = time.time()

    out = out.astype(np.float32).reshape(B, S, HID)
    t4 = time.time()
    print(f"[kernel] build={t1 - t0:.2f}s prep={t2 - t1:.2f}s "
          f"run={t3 - t2:.2f}s post={t4 - t3:.2f}s", file=sys.stderr)
    return out


# revision 7
# speedup vs baseline: 7.8699x; 1.6657x over previous
import sys
import time
import numpy as np

sys.path.insert(0, '/opt/trn_rl_repo')

import concourse.bass as bass
import concourse.bacc as bacc
import concourse.tile as tile
from concourse import mybir
from concourse.bass_utils import run_bass_kernel_spmd
from contextlib import ExitStack

F32 = mybir.dt.float32
F16 = mybir.dt.float16

B, S, HID = 2, 4096, 4096
NH, HD = 16, 256
RD = 64
THETA = 10000.0
T = B * S            # 8192 flat tokens
TPC = T // 8         # 1024 tokens per core
NEG = -30000.0
NHID = TPC * HID
NWQ = 1536 * HID
NWO = HID * 512
NCS = 64 * 128 * 32
NMS = 128 * 2048
NID = 128 * 128
NTOT = NHID + NWQ + NWO + 2 * NCS + NMS + NID

_cached = {}


def _build_program():
    nc = bacc.Bacc("TRN2", target_bir_lowering=False, debug=False, num_devices=8)
    # per-core inputs, all fp16 on the wire:
    #   hid:  this core's 1024-token slice of flattened hidden [T, HID]
    #   wqkv: rows [q(h0) q(h1) k(h0) k(h1) v(h0) v(h1)] x 256 for its 2 heads
    #   woutN: Wout[:, 512c:512c+512] (natural layout, transposed on device)
    blob_e = nc.declare_dram_parameter("blob", [NTOT], F16, isOutput=False)
    out_e = nc.declare_dram_parameter("out", [TPC, HID], F16, isOutput=True)
    o = 0
    hid_a = blob_e.ap()[o:o + NHID].rearrange("(t h) -> t h", h=HID); o += NHID
    wqkv_a = blob_e.ap()[o:o + NWQ].rearrange("(r h) -> r h", h=HID); o += NWQ
    wout_a = blob_e.ap()[o:o + NWO].rearrange("(r d) -> r d", d=512); o += NWO
    cs_a = blob_e.ap()[o:o + NCS].rearrange("(a p f) -> a p f", p=128, f=32); o += NCS
    sn_a = blob_e.ap()[o:o + NCS].rearrange("(a p f) -> a p f", p=128, f=32); o += NCS
    msk_a = blob_e.ap()[o:o + NMS].rearrange("(p f) -> p f", f=2048); o += NMS
    id_a = blob_e.ap()[o:o + NID].rearrange("(p q) -> p q", q=128); o += NID
    assert o == NTOT

    Copy = mybir.ActivationFunctionType.Copy
    Exp = mybir.ActivationFunctionType.Exp
    AX = mybir.AxisListType.X

    with tile.TileContext(nc) as tc:
        with tc.tile_pool(name="dram", bufs=1, space="DRAM") as dram, \
             tc.tile_pool(name="consts", bufs=1) as consts:
            hTs = dram.tile([HID, TPC], F16)       # hidden^T, my token slice
            gt = dram.tile([8, HID, TPC], F16)     # allgathered hidden^T
            QT = dram.tile([512, T], F16)          # q^T for my 2 heads (rope'd)
            KT = dram.tile([512, T], F16)
            VN = dram.tile([T, 512], F16)          # v, natural [token, d]
            AT = dram.tile([512, T], F16)          # attn out^T for my 2 heads
            PO = dram.tile([T, HID], F16)          # partial out-proj
            RSo = dram.tile([TPC, HID], F16)       # reduce-scattered slice

            idt = consts.tile([128, 128], F16, name="idt", tag="idt")
            nc.sync.dma_start(out=idt, in_=id_a)
            csf = consts.tile([128, 64, 32], F32, name="csf", tag="csf")
            snf = consts.tile([128, 64, 32], F32, name="snf", tag="snf")
            mskf = consts.tile([128, 2048], F32, name="mskf", tag="mskf")

            # ---------- phase 0: transpose own hidden slice, allgather ------
            with ExitStack() as s0:
                hin = s0.enter_context(tc.tile_pool(name="hin", bufs=2))
                hout = s0.enter_context(tc.tile_pool(name="hout", bufs=2))
                pst0 = s0.enter_context(tc.tile_pool(name="pst0", bufs=4, space="PSUM"))
                hTv = hTs.rearrange("(kc p) t -> p kc t", p=128)
                for tt in range(8):
                    hs = hin.tile([128, HID], F16, name="hs")
                    nc.sync.dma_start(out=hs, in_=hid_a[tt * 128:(tt + 1) * 128, :])
                    hb = hout.tile([128, 32, 128], F16, name="hb")
                    for kc in range(32):
                        tp = pst0.tile([128, 128], F16, name="tp0")
                        nc.tensor.transpose(tp, hs[:, kc * 128:(kc + 1) * 128], idt)
                        nc.vector.tensor_copy(hb[:, kc, :], tp)
                    nc.sync.dma_start(out=hTv[:, :, tt * 128:(tt + 1) * 128], in_=hb)
                nc.gpsimd.collective_compute(
                    "AllGather", mybir.AluOpType.bypass,
                    replica_groups=[list(range(8))],
                    ins=[hTs[:]], outs=[gt[:]])

            # ---------- phase 1: QKV projection + RoPE + transposes ---------
            with ExitStack() as s1:
                wq = s1.enter_context(tc.tile_pool(name="wq", bufs=1))
                wn = s1.enter_context(tc.tile_pool(name="wn", bufs=2))
                hstr = s1.enter_context(tc.tile_pool(name="hstr", bufs=2))
                ev = s1.enter_context(tc.tile_pool(name="ev", bufs=4))
                tr = s1.enter_context(tc.tile_pool(name="tr", bufs=4))
                pmm = s1.enter_context(tc.tile_pool(name="pmm", bufs=2, space="PSUM"))
                ptr = s1.enter_context(tc.tile_pool(name="ptr", bufs=4, space="PSUM"))

                # load + upcast cos/sin/mask constants
                cst = ev.tile([128, 64, 32], F16, name="cst", bufs=1)
                nc.sync.dma_start(out=cst, in_=cs_a.rearrange("tt p f -> p tt f"))
                nc.scalar.activation(csf.rearrange("p a b -> p (a b)"),
                                     cst.rearrange("p a b -> p (a b)"), Copy)
                snt = ev.tile([128, 64, 32], F16, name="snt", bufs=1)
                nc.sync.dma_start(out=snt, in_=sn_a.rearrange("tt p f -> p tt f"))
                nc.scalar.activation(snf.rearrange("p a b -> p (a b)"),
                                     snt.rearrange("p a b -> p (a b)"), Copy)
                mskst = ev.tile([128, 2048], F16, name="mskst", bufs=1)
                nc.sync.dma_start(out=mskst, in_=msk_a)
                nc.scalar.activation(mskf, mskst, Copy)

                # device-side transpose of wqkv -> 32 resident [128k, 1536o]
                wqkvT = [wq.tile([128, 1536], F16, name=f"wt{kc}", tag=f"wt{kc}")
                         for kc in range(32)]
                for j in range(12):
                    wnat = wn.tile([128, HID], F16, name="wnat")
                    nc.sync.dma_start(out=wnat,
                                      in_=wqkv_a[j * 128:(j + 1) * 128, :])
                    for kc in range(32):
                        tp = ptr.tile([128, 128], F16, name="tp1")
                        nc.tensor.transpose(tp, wnat[:, kc * 128:(kc + 1) * 128], idt)
                        nc.vector.tensor_copy(wqkvT[kc][:, j * 128:(j + 1) * 128], tp)

                gv = gt.rearrange("blk (kc p) t -> blk p kc t", p=128)
                for tt in range(64):
                    blk, ts = tt // 8, (tt % 8) * 128
                    hT = hstr.tile([128, 32, 128], F16, name="hT")
                    nc.sync.dma_start(out=hT, in_=gv[blk, :, :, ts:ts + 128])
                    for oc in range(3):
                        ps = pmm.tile([128, 512], F32, name="qkvps")
                        for kc in range(32):
                            nc.tensor.matmul(
                                ps, hT[:, kc, :],
                                wqkvT[kc][:, oc * 512:(oc + 1) * 512],
                                start=(kc == 0), stop=(kc == 31))
                        ot = ev.tile([128, 512], F16, name="ot")
                        if oc < 2:
                            # GPT-J interleaved rope on first 64 dims per head;
                            # rotated pairs written deinterleaved (blocks of 32)
                            # -- ok since q and k get the same permutation.
                            for h in range(2):
                                b0 = h * 256
                                x1 = ps[:, b0 + 0:b0 + 64:2]
                                x2 = ps[:, b0 + 1:b0 + 65:2]
                                ct = csf[:, tt, :]
                                st_ = snf[:, tt, :]
                                ta = tr.tile([128, 32], F32, name="ta")
                                tb = tr.tile([128, 32], F32, name="tb")
                                nc.vector.tensor_mul(ta, x1, ct)
                                nc.vector.tensor_mul(tb, x2, st_)
                                nc.vector.tensor_sub(ot[:, b0:b0 + 32], ta, tb)
                                tc2 = tr.tile([128, 32], F32, name="tc2")
                                td = tr.tile([128, 32], F32, name="td")
                                nc.vector.tensor_mul(tc2, x2, ct)
                                nc.vector.tensor_mul(td, x1, st_)
                                nc.vector.tensor_add(ot[:, b0 + 32:b0 + 64], tc2, td)
                                nc.scalar.activation(ot[:, b0 + 64:b0 + 256],
                                                     ps[:, b0 + 64:b0 + 256], Copy)
                            dst = QT if oc == 0 else KT
                            for db in range(4):
                                tp = ptr.tile([128, 128], F16, name="tp1")
                                nc.tensor.transpose(tp, ot[:, db * 128:(db + 1) * 128], idt)
                                ob = ev.tile([128, 128], F16, name="ob")
                                nc.vector.tensor_copy(ob, tp)
                                nc.sync.dma_start(
                                    out=dst[db * 128:(db + 1) * 128,
                                            tt * 128:(tt + 1) * 128],
                                    in_=ob)
                        else:
                            nc.scalar.activation(ot, ps, Copy)
                            nc.sync.dma_start(
                                out=VN[tt * 128:(tt + 1) * 128, :], in_=ot)

            # ---------- phase 2: causal attention for my 2 heads ------------
            with ExitStack() as s2:
                kvp = s2.enter_context(tc.tile_pool(name="kvp", bufs=2))
                pts = s2.enter_context(tc.tile_pool(name="pts", bufs=1))
                sp = s2.enter_context(tc.tile_pool(name="sp", bufs=2))
                sm = s2.enter_context(tc.tile_pool(name="sm", bufs=4))
                aot = s2.enter_context(tc.tile_pool(name="aot", bufs=3))
                pss = s2.enter_context(tc.tile_pool(name="pss", bufs=2, space="PSUM"))
                pso = s2.enter_context(tc.tile_pool(name="pso", bufs=1, space="PSUM"))
                ptp = s2.enter_context(tc.tile_pool(name="ptp", bufs=4, space="PSUM"))
                vv = VN.rearrange("(g p) d -> p g d", p=128)
                for h in range(2):
                    for b in range(2):
                        q2, k2 = [], []
                        for d in range(2):
                            qt_ = kvp.tile([128, S], F16, name=f"qt{d}")
                            nc.sync.dma_start(
                                out=qt_,
                                in_=QT[h * 256 + d * 128:h * 256 + (d + 1) * 128,
                                       b * S:(b + 1) * S])
                            q2.append(qt_)
                            kt_ = kvp.tile([128, S], F16, name=f"kt{d}")
                            nc.sync.dma_start(
                                out=kt_,
                                in_=KT[h * 256 + d * 128:h * 256 + (d + 1) * 128,
                                       b * S:(b + 1) * S])
                            k2.append(kt_)
                        vt = kvp.tile([128, 32, 256], F16, name="vt", bufs=1)
                        nc.sync.dma_start(
                            out=vt, in_=vv[:, b * 32:(b + 1) * 32,
                                           h * 256:(h + 1) * 256])
                        for qb in range(8):
                            nk = qb + 1
                            pt_t = pts.tile([128, 32, 512], F16, name="ptt")
                            for qs in range(4):
                                qo = qb * 512 + qs * 128
                                prow = sp.tile([128, 4096], F16, name="prow")
                                sums = sm.tile([128, 8], F32, name="sums")
                                for kc in range(nk):
                                    ps_ = pss.tile([128, 512], F32, name="sps")
                                    for d in range(2):
                                        nc.tensor.matmul(
                                            ps_, q2[d][:, qo:qo + 128],
                                            k2[d][:, kc * 512:(kc + 1) * 512],
                                            start=(d == 0), stop=(d == 1))
                                    if kc == qb:
                                        srow = sm.tile([128, 512], F32, name="srow")
                                        nc.vector.tensor_add(
                                            srow, ps_,
                                            mskf[:, qs * 512:(qs + 1) * 512])
                                        nc.scalar.activation(
                                            prow[:, kc * 512:(kc + 1) * 512],
                                            srow, Exp, scale=1.0 / 16.0,
                                            accum_out=sums[:, kc:kc + 1])
                                    else:
                                        nc.scalar.activation(
                                            prow[:, kc * 512:(kc + 1) * 512],
                                            ps_, Exp, scale=1.0 / 16.0,
                                            accum_out=sums[:, kc:kc + 1])
                                ssum = sm.tile([128, 1], F32, name="ssum")
                                nc.vector.reduce_sum(ssum, sums[:, 0:nk], axis=AX)
                                rinv = sm.tile([128, 1], F32, name="rinv")
                                nc.vector.reciprocal(rinv, ssum)
                                pscl = sp.tile([128, 4096], F16, name="pscl")
                                nc.vector.tensor_scalar_mul(
                                    pscl[:, 0:nk * 512], prow[:, 0:nk * 512], rinv)
                                for g in range(nk * 4):
                                    tp = ptp.tile([128, 128], F16, name="ptp")
                                    nc.tensor.transpose(
                                        tp, pscl[:, g * 128:(g + 1) * 128], idt)
                                    nc.vector.tensor_copy(
                                        pt_t[:, g, qs * 128:(qs + 1) * 128], tp)
                            po2 = [pso.tile([128, 512], F32, name=f"po{d}")
                                   for d in range(2)]
                            for g in range(nk * 4):
                                for d in range(2):
                                    nc.tensor.matmul(
                                        po2[d], vt[:, g, d * 128:(d + 1) * 128],
                                        pt_t[:, g, :],
                                        start=(g == 0), stop=(g == nk * 4 - 1))
                            for d in range(2):
                                ao = aot.tile([128, 512], F16, name="ao")
                                nc.scalar.activation(ao, po2[d], Copy)
                                nc.sync.dma_start(
                                    out=AT[h * 256 + d * 128:h * 256 + (d + 1) * 128,
                                           b * S + qb * 512:b * S + (qb + 1) * 512],
                                    in_=ao)

            # ---------- phase 3: output projection + reduce-scatter ---------
            with ExitStack() as s3:
                wo4 = s3.enter_context(tc.tile_pool(name="wo4", bufs=1))
                wos = s3.enter_context(tc.tile_pool(name="wos", bufs=2))
                ap_ = s3.enter_context(tc.tile_pool(name="ap", bufs=2))
                ob_ = s3.enter_context(tc.tile_pool(name="obp", bufs=3))
                pf = s3.enter_context(tc.tile_pool(name="pf", bufs=2, space="PSUM"))
                ptw = s3.enter_context(tc.tile_pool(name="ptw", bufs=4, space="PSUM"))
                w4 = wo4.tile([128, 4, HID], F16, name="w4", tag="w4")
                for j in range(32):
                    wns = wos.tile([128, 512], F16, name="wns")
                    nc.sync.dma_start(out=wns,
                                      in_=wout_a[j * 128:(j + 1) * 128, :])
                    for dc in range(4):
                        tp = ptw.tile([128, 128], F16, name="wtp2")
                        nc.tensor.transpose(tp, wns[:, dc * 128:(dc + 1) * 128], idt)
                        nc.vector.tensor_copy(w4[:, dc, j * 128:(j + 1) * 128], tp)
                atv = AT.rearrange("(dc p) t -> p dc t", p=128)
                for tt in range(64):
                    at = ap_.tile([128, 4, 128], F16, name="at")
                    nc.sync.dma_start(out=at, in_=atv[:, :, tt * 128:(tt + 1) * 128])
                    oto = ob_.tile([128, HID], F16, name="oto")
                    for oc in range(8):
                        ps2 = pf.tile([128, 512], F32, name="ps2")
                        for dc in range(4):
                            nc.tensor.matmul(
                                ps2, at[:, dc, :],
                                w4[:, dc, oc * 512:(oc + 1) * 512],
                                start=(dc == 0), stop=(dc == 3))
                        nc.scalar.activation(oto[:, oc * 512:(oc + 1) * 512], ps2, Copy)
                    nc.sync.dma_start(out=PO[tt * 128:(tt + 1) * 128, :], in_=oto)
                nc.gpsimd.collective_compute(
                    "ReduceScatter", mybir.AluOpType.add,
                    replica_groups=[list(range(8))],
                    ins=[PO[:]], outs=[RSo[:]])
                for i in range(8):
                    t_ = ob_.tile([128, HID], F16, name="cpy", bufs=2)
                    nc.sync.dma_start(out=t_, in_=RSo[i * 128:(i + 1) * 128, :])
                    nc.sync.dma_start(out=out_e.ap()[i * 128:(i + 1) * 128, :], in_=t_)

    nc.compile()
    return nc


def _make_runner(nc):
    """Build a cached jitted executor for nc (trace/lower once, reuse)."""
    import jax
    import jax.numpy as jnp
    from jax.sharding import Mesh, PartitionSpec, NamedSharding
    try:
        from jax.experimental.shard_map import shard_map
    except ImportError:
        from jax import shard_map
    from concourse import bass2jax as b2j

    b2j.install_neuronx_cc_hook()
    assert nc.dbg_addr is None
    partition_name = nc.partition_id_tensor.name if nc.partition_id_tensor else None
    in_names, out_names, out_avals = [], [], []
    for alloc in nc.m.functions[0].allocations:
        if not isinstance(alloc, mybir.MemoryLocationSet):
            continue
        name = alloc.memorylocations[0].name
        if alloc.kind == "ExternalInput":
            if name != partition_name:
                in_names.append(name)
        elif alloc.kind == "ExternalOutput":
            out_names.append(name)
            shape = tuple(alloc.tensor_shape)
            dtype = mybir.dt.np(alloc.dtype)
            out_avals.append(jax.core.ShapedArray(shape, dtype))
    n_params = len(in_names)
    all_names = tuple(in_names + out_names +
                      ([partition_name] if partition_name else []))
    donate = tuple(range(n_params, n_params + len(out_names)))

    def _body(*args):
        operands = list(args)
        if partition_name is not None:
            operands.append(b2j.partition_id_tensor())
        outs = b2j._bass_exec_p.bind(
            *operands, out_avals=tuple(out_avals), in_names=all_names,
            out_names=tuple(out_names), lowering_input_output_aliases=(),
            sim_require_finite=True, sim_require_nnan=True, nc=nc)
        return tuple(outs)

    devices = jax.devices()[:8]
    mesh = Mesh(np.asarray(devices), ("core",))
    spec = PartitionSpec("core")
    sharded = jax.jit(
        shard_map(_body, mesh=mesh,
                  in_specs=(spec,) * (n_params + len(out_names)),
                  out_specs=(spec,) * len(out_names), check_rep=False),
        donate_argnums=donate, keep_unused=True)
    sh = NamedSharding(mesh, spec)
    zero_fns = [
        jax.jit(lambda a=a: jnp.zeros((8 * a.shape[0],) + tuple(a.shape[1:]),
                                      a.dtype), out_shardings=sh)
        for a in out_avals]

    from concurrent.futures import ThreadPoolExecutor

    def run(pack_fn):
        # pack_fn(c) -> 1-D np.float16 blob for core c; puts overlap packing
        tA = time.time()
        assert in_names == ["blob"], in_names
        shards = []
        for c in range(8):
            shards.append(jax.device_put(pack_fn(c), devices[c]))
        gins = [jax.make_array_from_single_device_arrays(
            (8 * shards[0].shape[0],), sh, shards)]
        zeros = [zf() for zf in zero_fns]
        for g in gins:
            g.block_until_ready()
        tB = time.time()
        outs = sharded(*gins, *zeros)
        for o in outs:
            o.block_until_ready()
        tC = time.time()
        out_f32 = np.empty((T, HID), np.float32)
        def grab(shard):
            out_f32[shard.index] = np.asarray(shard.data)
        with ThreadPoolExecutor(8) as ex:
            list(ex.map(grab, outs[0].addressable_shards))
        tD = time.time()
        print(f"[runner] put+pack={tB - tA:.2f}s exec={tC - tB:.2f}s "
              f"fetch={tD - tC:.2f}s", file=sys.stderr)
        return out_f32

    return run


def kernel(hidden_states, position_ids, Wqkv, Wout):
    t0 = time.time()
    hs = np.asarray(hidden_states, dtype=np.float32).reshape(T, HID)
    pos = np.asarray(position_ids).reshape(T).astype(np.float32)
    Wqkv = np.asarray(Wqkv, dtype=np.float32)
    Wout = np.asarray(Wout, dtype=np.float32)

    if "nc" not in _cached:
        _cached["nc"] = _build_program()
    nc = _cached["nc"]
    t1 = time.time()

    inv_freq = (1.0 / (THETA ** (np.arange(0, RD, 2, dtype=np.float64) / RD))
                ).astype(np.float32)
    fr = pos[:, None] * inv_freq[None, :]
    cs16 = np.cos(fr).astype(np.float16).ravel()
    sn16 = np.sin(fr).astype(np.float16).ravel()
    rr = np.arange(128)[:, None]
    cc = np.arange(512)[None, :]
    msk16 = np.concatenate([np.where(cc <= 128 * q + rr, 0.0, NEG)
                            for q in range(4)], axis=1).astype(np.float16).ravel()
    id16 = np.eye(128, dtype=np.float16).ravel()
    wq3 = Wqkv.reshape(3, 8, 512, HID)

    if "blob" not in _cached:
        _cached["blob"] = np.empty((8, NTOT), dtype=np.float16)
    blob = _cached["blob"]
    O0, O1, O2, O3, O4, O5 = (NHID, NHID + NWQ, NHID + NWQ + NWO,
                              NHID + NWQ + NWO + NCS,
                              NHID + NWQ + NWO + 2 * NCS,
                              NHID + NWQ + NWO + 2 * NCS + NMS)

    def pack(c):
        b = blob[c]
        np.copyto(b[:O0].reshape(TPC, HID), hs[c * TPC:(c + 1) * TPC])
        np.copyto(b[O0:O1].reshape(3, 512, HID), wq3[:, c])
        np.copyto(b[O1:O2].reshape(HID, 512), Wout[:, c * 512:(c + 1) * 512])
        b[O2:O3] = cs16
        b[O3:O4] = sn16
        b[O4:O5] = msk16
        b[O5:] = id16
        return b
    t2 = time.time()

    try:
        if "runner" not in _cached:
            _cached["runner"] = _make_runner(nc)
        out = _cached["runner"](pack)
    except Exception as e:
        print(f"[kernel] cached runner failed ({e!r}); falling back",
              file=sys.stderr)
        _cached.pop("runner", None)
        in_maps = [{"blob": pack(c).copy()} for c in range(8)]
        res = run_bass_kernel_spmd(nc, in_maps, list(range(8))).results
        out = np.concatenate([res[c]["out"] for c in range(8)],
                             axis=0).astype(np.float32)
    t3 = time.time()

    out = out.reshape(B, S, HID)
    t4 = time.time()
    print(f"[kernel] build={t1 - t0:.2f}s prep={t2 - t1:.2f}s "
          f"run={t3 - t2:.2f}s post={t4 - t3:.2f}s", file=sys.stderr)
    return out
